# revision 44
# baseline (speedup 1.0000x reference)
"""Multi-head cross-attention on Trainium2, 8-core SPMD.

Problem (hardcoded): B=4, T=2048, D=1024, H=16 heads, head_dim=64, fp32.
    kv = x_enc @ Wkv + bkv ; q = x_dec @ Wq + bq
    per head: S = q_h k_h^T / sqrt(64); P = softmax(S + mask); O_h = P v_h
    out = concat_h(O_h) @ Wo + bo

Sharding: data parallel over batch (4 slices x 2 cores each) and tensor
parallel over heads within each pair (8 heads per core).  Each core
computes a partial output  Y_local @ Wo[rows_local]  (+bo on the even
core of the pair); the host unshards by summing each pair's partials
and stacking the 4 batch slices.  The host pre-transposes the
activations, regroups Wkv columns, and converts x/W inputs to bf16
(all matmul operands run bf16 with fp32 PSUM accumulation; measured
rel err ~4e-3 vs the 2e-2 gate).

The mask input is structurally zero for this problem (spec fill
"zeros"); softmax(S + 0) == softmax(S), so the kernel does not load it
(checked on the host).

Per-core schedule — one fused, software-pipelined stream:
  * Pre-attention (shortest path to starting the ACT exp stream):
    K^T for pair 0, V for all k-tiles in NATURAL [k, d] layout
    (stationary = x_enc^T chunks; per-head [k, 65] layout whose 65th
    column is a constant 1 -> softmax denominators fall out of the PV
    matmul for free), and Q^T for q-chunk 0.
  * Attention: ONE flat pipeline over all 256 (q-chunk, pair, k-tile)
    steps: S^T tile = (K^T slice)^T Q^T slice per parity (64-partition
    contraction, no padding); P^T = exp(S^T/8) on ACT ([128,1024] per
    instruction, bf16 out) lagging mm1 by 2; O'^T += V_aug^T P^T
    lagging exp by 2 (pt ring depth 5) so the PE never waits on the
    exp just issued.  The attention body is ACT-bound (~1.0us/step);
    all remaining work drips into the PE stream as filler:
      - the remaining K^T/Q^T projection matmuls (one per step,
        force-drained before any consumer so program order is safe),
      - the previous q-chunk's output projection (one matmul per odd
        step; Y^T as stationary vs Wo tiles, +bo).
  * Pair-boundary normalize, off the critical path: the PSUM
    accumulators are copied to SBUF scratch immediately (fast free),
    the denominator row is DMA'd to partition 0, reciprocal on DVE
    (approx, 18 bits), partition-broadcast on the otherwise-idle
    GpSimd engine, multiply into the pair-stacked Y^T tile (odd
    parity partition-shifted via SBUF->SBUF DMA).
  PSUM: S^T 2x[128,1024] (4 banks) + PV accumulators (2) + shared
  aux ring for projections/out-projection (2) = 8 banks exactly.
"""

import numpy as np

import concourse.bass as bass
import concourse.mybir as mybir
import concourse.tile as tile
from concourse import bacc
from concourse.bass_utils import run_bass_kernel_spmd
from concourse.masks import make_identity

f32 = mybir.dt.float32
f32r = mybir.dt.float32r
bf16 = mybir.dt.bfloat16
AF = mybir.ActivationFunctionType
ALU = mybir.AluOpType

P = 128


def build_nc(T=2048, D=1024, HPC=8, HD=64, n_cores=8, use_f32r=True):
    """Build + compile the per-core Bass program. HPC = heads per core."""
    assert HD == 64 and HPC % 2 == 0 and T % 512 == 0 and D % P == 0
    CPC = HPC * HD          # q/out channels per core (512)
    TC = 512                # token chunk (psum free dim), phases A/B
    QC = 512                # q chunk width, attention phase
    NQ = T // TC            # token chunks (4)
    ND = D // P             # model-dim chunks (8)
    NG = HPC // 2           # head pairs (4)
    NKT = T // P            # k-token tiles (16)
    NQC = T // QC           # q chunks (4)
    HD1 = HD + 1            # V columns + ones column (65)
    HD2 = HD + 4            # padded to 68 cols: 8-byte-aligned bf16
                            # stationary slices (65*2B is 2-byte aligned
                            # and measurably slows LDWEIGHTS)
    VH = NKT * HD2          # per-head vnat columns
    SCALE = float(1.0 / np.sqrt(HD))
    ON = 512                # out-proj free chunk
    NON = D // ON           # 2

    MDT = f32r if use_f32r else f32    # dtype of matmul-input tiles
    BDT = bf16                         # dtype of attention operands

    def rr_(ap):                       # bitcast for DMA loads from f32 DRAM
        return ap.bitcast(MDT) if use_f32r else ap

    nc = bacc.Bacc("TRN2", target_bir_lowering=False, debug=False,
                   enable_asserts=False, num_devices=n_cores)

    xeT = nc.dram_tensor("x_enc_t", [D, T], bf16, kind="ExternalInput").ap()
    xdT = nc.dram_tensor("x_dec_t", [D, T], bf16, kind="ExternalInput").ap()
    wq_d = nc.dram_tensor("wq", [D, CPC], bf16, kind="ExternalInput").ap()
    wkv_d = nc.dram_tensor("wkv_g", [D, 2 * CPC], bf16, kind="ExternalInput").ap()
    wo_d = nc.dram_tensor("wo", [CPC, D], bf16, kind="ExternalInput").ap()
    bq_d = nc.dram_tensor("bq", [CPC], f32, kind="ExternalInput").ap()
    bkv_d = nc.dram_tensor("bkv_g", [2 * CPC], f32, kind="ExternalInput").ap()
    bo_d = nc.dram_tensor("bo", [D], f32, kind="ExternalInput").ap()
    out_d = nc.dram_tensor("out", [T, D], f32, kind="ExternalOutput").ap()

    with tile.TileContext(nc) as tc:
      with tc.tile_pool(name="const", bufs=1) as cpool:
        # identity (ones source) + ones rows for PE broadcasts: row 0
        # (bias rows at partition 0); f32r producers must be DVE ops
        ident = cpool.tile([P, P], f32, name="ident")
        make_identity(nc, ident)
        ones_t = cpool.tile([P, P], f32, name="ones_t")
        nc.vector.tensor_scalar(ones_t[0:1, :], ident[0:1, :],
                                0.0, 1.0, ALU.mult, ALU.add)

        bo_row = cpool.tile([1, D], f32, name="bo_row")
        bkv_k_sb = cpool.tile([P, NG], f32, name="bkv_k_sb")
        bq_sb = cpool.tile([P, NG], f32, name="bq_sb")
        bkv_v_row = cpool.tile([1, CPC], f32, name="bkv_v_row")
        nc.sync.dma_start(out=bo_row[:], in_=bo_d[:].unsqueeze(0))
        nc.sync.dma_start(out=bkv_v_row[:],
                          in_=bkv_d[CPC:2 * CPC].unsqueeze(0))

        # persistent activations
        kT = [cpool.tile([P, T], BDT, name=f"kT{g}") for g in range(NG)]
        qT = [cpool.tile([P, T], BDT, name=f"qT{g}") for g in range(NG)]
        # V natural, per head h: [k-token partitions, NKT*(HD+1)];
        # column kt*65+64 is constant 1.0 (softmax denominator trick)
        vnat = cpool.tile([P, HPC * VH], BDT, name="vnat")
        vnat3 = vnat[:].rearrange("p (h c) -> p h c", h=HPC)

        def vn(h):                      # per-head view [P, VH]
            return vnat3[:, h, :]

        wo_sb = [cpool.tile([P, D], BDT, name=f"wo{g}") for g in range(NG)]
        bo_bc = cpool.tile([P, D], f32, name="bo_bc")

        # ---------- Fused projections + attention + out-projection ----------
        # One scope: V + K(g0) + Q(qc0) are produced up front (shortest
        # path to starting the ACT exp stream); the remaining K/Q
        # projection matmuls and the out-projection are drip-fed into the
        # attention step loop as PE filler behind the ACT-bound exp stream.
        with tc.tile_pool(name="work", bufs=1) as wp, \
             tc.tile_pool(name="ps_S", bufs=1, space="PSUM") as psS, \
             tc.tile_pool(name="ps_O", bufs=1, space="PSUM") as psO, \
             tc.tile_pool(name="ps_aux", bufs=1, space="PSUM") as psA:

            xcs = [wp.tile([P, T], BDT, name=f"xc{d}") for d in range(ND)]
            xds = [wp.tile([P, T], BDT, name=f"xd{d}") for d in range(ND)]
            wkv_sb = [wp.tile([P, 2 * CPC], BDT, name=f"wkv{d}")
                      for d in range(ND)]
            wq_sb = [wp.tile([P, CPC], BDT, name=f"wq{d}")
                     for d in range(ND)]

            # DMA emission order = need order: tiny bias rows, x_enc +
            # wkv (critical for the pre-C work), x_dec q-chunk 0 + wq
            # (pre-C B), then the rest.
            nc.sync.dma_start(out=bkv_v_row[:],
                              in_=bkv_d[CPC:2 * CPC].unsqueeze(0))
            nc.sync.dma_start(out=bo_row[:], in_=bo_d[:].unsqueeze(0))
            for d in range(ND):      # wkv first: every group needs all d
                nc.sync.dma_start(out=wkv_sb[d][:, 0:CPC],
                                  in_=wkv_d[d * P:(d + 1) * P, 0:CPC])
                nc.sync.dma_start(out=wkv_sb[d][:, CPC:2 * CPC],
                                  in_=wkv_d[d * P:(d + 1) * P, CPC:2 * CPC])
            for c in range(4):       # x_enc in consumption order (chunk 0
                for d in range(ND):  # of every d-slice first)
                    cc = slice(c * (T // 4), (c + 1) * (T // 4))
                    nc.sync.dma_start(out=xcs[d][:, cc],
                                      in_=xeT[d * P:(d + 1) * P, cc])
            for g in range(NG):
                nc.sync.dma_start(out=bkv_k_sb[:, g:g + 1],
                                  in_=bkv_d[g * P:(g + 1) * P].unsqueeze(1))
                nc.sync.dma_start(out=bq_sb[:, g:g + 1],
                                  in_=bq_d[g * P:(g + 1) * P].unsqueeze(1))
            for d in range(ND):
                nc.sync.dma_start(out=xds[d][:, 0:TC],
                                  in_=xdT[d * P:(d + 1) * P, 0:TC])
                nc.sync.dma_start(out=wq_sb[d][:],
                                  in_=wq_d[d * P:(d + 1) * P, :])
            for d in range(ND):
                for c in range(1, 4):
                    cc = slice(c * (T // 4), (c + 1) * (T // 4))
                    nc.sync.dma_start(out=xds[d][:, cc],
                                      in_=xdT[d * P:(d + 1) * P, cc])
            for g in range(NG):
                nc.sync.dma_start(out=wo_sb[g][:, 0:CPC],
                                  in_=wo_d[g * P:(g + 1) * P, 0:CPC])
                nc.sync.dma_start(out=wo_sb[g][:, CPC:D],
                                  in_=wo_d[g * P:(g + 1) * P, CPC:D])

            # bias broadcast rows -> [128, *] (PE ones-matmul, fp32, tiny)
            bias_vbc = wp.tile([P, CPC], f32, name="bias_vbc")
            ps_vb = psA.tile([P, CPC], f32, tag="aux", bufs=2, name="ps_vb")
            nc.tensor.matmul(ps_vb[:], ones_t[0:1, :], bkv_v_row[0:1, :],
                             skip_group_check=True)
            nc.vector.tensor_copy(bias_vbc[:], ps_vb[:])
            # constant-1 denominator column (+ pad cols) of every V tile
            nc.vector.tensor_scalar(
                vnat[:].rearrange("p (h k c) -> p h k c", h=HPC, k=NKT)
                [:, :, :, HD:HD2],
                bias_vbc[:].rearrange("p (a b c) -> p a b c", a=HPC, b=NKT),
                0.0, 1.0, ALU.mult, ALU.add)
            for o in range(NON):
                ocols = slice(o * ON, (o + 1) * ON)
                ps_bo = psA.tile([P, ON], f32, tag="aux", bufs=2,
                                 name=f"psbo_{o}")
                nc.tensor.matmul(ps_bo[:], ones_t[0:1, :],
                                 bo_row[0:1, ocols], skip_group_check=True)
                nc.vector.tensor_copy(bo_bc[:, ocols], ps_bo[:])

            # ---- projection emitters (used dense pre-C, dripped in C) ----
            prj = {}

            def proj_mm(kind, g, tq, d):
                """One projection matmul: kind 'k' -> kT[g], 'q' -> qT[g]."""
                tcols = slice(tq * TC, (tq + 1) * TC)
                if d == 0:
                    prj[(kind, g, tq)] = psA.tile(
                        [P, TC], f32, tag="aux", bufs=2,
                        name=f"p{kind}_{g}_{tq}")
                pp = prj[(kind, g, tq)]
                if kind == "k":
                    nc.tensor.matmul(pp[:], wkv_sb[d][:, g * P:(g + 1) * P],
                                     xcs[d][:, tcols],
                                     start=(d == 0), stop=(d == ND - 1),
                                     skip_group_check=True)
                else:
                    nc.tensor.matmul(pp[:], wq_sb[d][:, g * P:(g + 1) * P],
                                     xds[d][:, tcols],
                                     start=(d == 0), stop=(d == ND - 1),
                                     skip_group_check=True)
                if d == ND - 1:
                    dst = kT[g] if kind == "k" else qT[g]
                    bias = bkv_k_sb if kind == "k" else bq_sb
                    nc.vector.tensor_scalar_add(
                        dst[:, tcols], prj.pop((kind, g, tq))[:],
                        bias[:, g:g + 1])

            vps = {}

            def v_mm(kt, d, tag="po"):
                """One V-natural matmul for 128-token tile kt (+evict)."""
                if d == 0:
                    pool = psO if tag == "po" else psA
                    vps[kt] = pool.tile([P, CPC], f32, tag=tag, bufs=2,
                                        name=f"pv_{kt}")
                pv = vps[kt]
                nc.tensor.matmul(
                    pv[:], xcs[d][:, kt * P:(kt + 1) * P],
                    wkv_sb[d][:, CPC:2 * CPC],
                    start=(d == 0), stop=(d == ND - 1),
                    skip_group_check=True)
                if d == ND - 1:
                    out3 = vnat[:].rearrange("p (h k c) -> p h k c",
                                             h=HPC, k=NKT)[:, :, kt, 0:HD]
                    nc.vector.tensor_tensor(
                        out3,
                        vps.pop(kt)[:].rearrange("p (h c) -> p h c", h=HPC),
                        bias_vbc[:].rearrange("p (h c) -> p h c", h=HPC),
                        ALU.add)

            emitted = set()

            # ---- pre-C: K(g0), V (all k-tiles), Q(qc0, all pairs) ----
            for tq in range(NQ):
                for d in range(ND):
                    proj_mm("k", 0, tq, d)
                emitted.add(("k", 0, tq))
                for i in range(TC // P):
                    kt = tq * (TC // P) + i
                    for d in range(ND):
                        v_mm(kt, d)
                    emitted.add(("v", kt, None))
            for g in range(NG):
                for d in range(ND):
                    proj_mm("q", g, 0, d)
                emitted.add(("q", g, 0))

            # ---- injected work queue (deadline order) ----
            inj = []

            def add_group(item8):
                kind, a, b = item8
                for d in range(ND):
                    inj.append((kind, a, b, d))

            for g in range(1, NG):
                for tq in range(NQ):
                    add_group(("k", g, tq))
            for tq in range(1, NQ):
                for g in range(NG):
                    add_group(("q", g, tq))

            # flat pipeline state
            steps = [(qc, g, kt)
                     for qc in range(NQC) for g in range(NG)
                     for kt in range(NKT)]
            NS = len(steps)
            pss = {}      # step idx -> psS tile
            pts = {}      # step idx -> pt tile
            pos = {}      # (qc, g) -> [po_e, po_o]
            yts = {}      # qc -> list of yT tiles per g
            osts = {}     # (qc, qt) -> ost tile
            pouts = {}

            def mm1_h(i, h2):
                qc, g, kt = steps[i]
                if h2 == 0:
                    force("k", g, kt // (TC // P))
                    force("q", g, qc)
                    pss[i] = psS.tile([P, 2 * QC], f32, tag="ps", bufs=2,
                                      name=f"ps_{i}")
                qcols = slice(qc * QC, (qc + 1) * QC)
                nc.tensor.matmul(
                    pss[i][:, h2 * QC:(h2 + 1) * QC],
                    kT[g][h2 * HD:(h2 + 1) * HD, kt * P:(kt + 1) * P],
                    qT[g][h2 * HD:(h2 + 1) * HD, qcols],
                    skip_group_check=True)

            def mm1(i):
                mm1_h(i, 0)
                mm1_h(i, 1)

            def do_exp(i):
                pt = wp.tile([P, 2 * QC], BDT, tag="pt", bufs=5,
                             name=f"pt_{i}")
                nc.scalar.activation(pt[:], pss.pop(i)[:], AF.Exp,
                                     scale=SCALE)
                pts[i] = pt

            def mm2_h(i, h2):
                qc, g, kt = steps[i]
                if h2 == 0:
                    force("v", kt, None)
                    if kt == 0:
                        pos[(qc, g)] = [
                            psO.tile([HD2, QC], f32, tag="po", bufs=2,
                                     name=f"po_{qc}_{g}_{hh}")
                            for hh in range(2)]
                po = pos[(qc, g)]
                pt = pts[i]
                nc.tensor.matmul(
                    po[h2][0:HD2, :],
                    vn(2 * g + h2)[:, kt * HD2:(kt + 1) * HD2],
                    pt[:, h2 * QC:(h2 + 1) * QC],
                    start=(kt == 0), stop=(kt == NKT - 1),
                    skip_group_check=True)
                if h2 == 1:
                    pts.pop(i)

            def mm2(i):
                mm2_h(i, 0)
                mm2_h(i, 1)

            def normalize(qc, g):
                """Evict+normalize pair (qc, g): frees po fast via SBUF
                scratch copies, then reciprocal/broadcast/mul off the
                critical path."""
                if g == 0:
                    yts[qc] = [wp.tile([P, QC], BDT, tag=f"yT{gg}", bufs=3,
                                       name=f"yT_{qc}_{gg}")
                               for gg in range(NG)]
                po = pos.pop((qc, g))
                scr = [wp.tile([HD1, QC], f32, tag=f"scr{h2}", bufs=1,
                               name=f"scr_{qc}_{g}_{h2}") for h2 in range(2)]
                # fast po eviction (DVE), even parity first
                for h2 in range(2):
                    nc.vector.tensor_copy(scr[h2][:], po[h2][0:HD1, :])
                # GpSimd partition_broadcast reads its source on Q7 core 0
                # (partitions 0-15 only): move the denominator row from
                # partition 64 to partition 0 via a tiny SBUF->SBUF DMA
                den = [wp.tile([1, QC], f32, tag=f"den{h2}", bufs=1,
                               name=f"den_{qc}_{g}_{h2}") for h2 in range(2)]
                rr = [wp.tile([1, QC], f32, tag=f"rr{h2}", bufs=1,
                              name=f"rr_{qc}_{g}_{h2}") for h2 in range(2)]
                for h2 in range(2):
                    nc.sync.dma_start(out=den[h2][0:1, :],
                                      in_=scr[h2][HD:HD1, :])
                for h2 in range(2):
                    nc.vector.reciprocal_approx_fast(
                        out=rr[h2][0:1, :], in_=den[h2][0:1, :])
                # broadcast the reciprocal row to 64 partitions on the (idle)
                # GpSimd engine; keeps the whole chain off PE/PSUM
                rbc = [wp.tile([HD, QC], f32, tag=f"rbc{h2}", bufs=1,
                               name=f"rbc_{qc}_{g}_{h2}") for h2 in range(2)]
                for h2 in range(2):
                    nc.gpsimd.partition_broadcast(
                        rbc[h2][:], rr[h2][0:1, :], channels=HD)
                # even parity -> yT rows 0:64 directly
                nc.vector.tensor_tensor(
                    yts[qc][g][0:HD, :],
                    scr[0][0:HD, :], rbc[0][:], ALU.mult)
                # odd parity -> staging tile, DMA shift to rows 64:128
                stg = wp.tile([HD, QC], BDT, tag="stg", bufs=2,
                              name=f"stg_{qc}_{g}")
                nc.vector.tensor_tensor(
                    stg[:],
                    scr[1][0:HD, :], rbc[1][:], ALU.mult)
                nc.sync.dma_start(out=yts[qc][g][HD:P, :], in_=stg[:])

            def outproj_mm(qc, qt, o, g):
                """One out-projection matmul (of an NG-deep accumulation
                group) for q-subtile qt, column chunk o, of q-chunk qc."""
                ocols = slice(o * ON, (o + 1) * ON)
                if o == 0 and g == 0:
                    osts[(qc, qt)] = wp.tile([P, D], f32, tag="ost", bufs=1,
                                             name=f"ost_{qc}_{qt}")
                if g == 0:
                    pouts[(qc, qt, o)] = psA.tile([P, ON], f32, tag="aux",
                                                  bufs=2,
                                                  name=f"pout_{qc}_{qt}_{o}")
                pout = pouts[(qc, qt, o)]
                nc.tensor.matmul(
                    pout[:], yts[qc][g][:, qt * P:(qt + 1) * P],
                    wo_sb[g][:, ocols],
                    start=(g == 0), stop=(g == NG - 1),
                    skip_group_check=True)
                if g == NG - 1:
                    ost = osts[(qc, qt)]
                    nc.vector.tensor_add(ost[:, ocols],
                                         pouts.pop((qc, qt, o))[:],
                                         bo_bc[:, ocols])
                    if o == NON - 1:
                        row0 = (qc * (QC // P) + qt) * P
                        nc.sync.dma_start(out=out_d[row0:row0 + P, :],
                                          in_=osts.pop((qc, qt))[:])

            def emit_inj(it):
                if it[0] == "v":
                    v_mm(it[1], it[3], tag="aux")
                elif it[0] in ("k", "q"):
                    proj_mm(it[0], it[1], it[2], it[3])
                else:
                    outproj_mm(*it[1:])
                if it[0] in ("k", "q", "v") and it[3] == ND - 1:
                    emitted.add((it[0], it[1], it[2]))

            def force(kind, a, b):
                """Emit queued work until group (kind, a, b) is complete —
                guarantees producers precede consumers in program order."""
                while (kind, a, b) not in emitted:
                    emit_inj(inj.pop(0))

            def drain(kt):
                # even steps: one projection matmul (deadline misses are
                # repaired by force()); odd steps: one out-projection
                # matmul.  Keeps out-projection from queueing up behind
                # the projections and landing in the drain tail.
                if inj and (inj[0][0] in ("k", "q", "v") or kt % 2 == 1):
                    emit_inj(inj.pop(0))

            def post_mm2(j):
                qc, g, kt = steps[j]
                if kt == NKT - 1:
                    normalize(qc, g)
                    if g == NG - 1:
                        for qt in range(QC // P):
                            for o in range(NON):
                                for gg in range(NG):
                                    inj.append(("o", qc, qt, o, gg))

            # mm2 lags exp by 2 steps (pt ring depth 4) so the PE never
            # waits on the exp just issued; mm1 leads by 3.
            mm1(0)
            mm1(1)
            do_exp(0)
            mm1(2)
            do_exp(1)
            for i in range(NS):
                if i >= 1:
                    mm2(i - 1)
                if i + 3 < NS:
                    mm1(i + 3)
                if i + 2 < NS:
                    do_exp(i + 2)
                if i >= 1:
                    post_mm2(i - 1)
                drain(steps[i][2])
            mm2(NS - 1)
            post_mm2(NS - 1)
            while inj:
                emit_inj(inj.pop(0))

    nc.compile()
    return nc


# ---------------------------------------------------------------------------
# Host side: sharding, run, unshard
# ---------------------------------------------------------------------------

_NC_CACHE = {}


def _get_nc():
    key = "full"
    if key not in _NC_CACHE:
        _NC_CACHE[key] = build_nc()
    return _NC_CACHE[key]


def _group_kv_cols(w_slice, HPC, HD):
    """Reorder kv columns [h-major, (k|v), d] -> K head-pair groups then V."""
    last = w_slice.shape[-1]
    assert last == HPC * 2 * HD
    arr = w_slice.reshape(w_slice.shape[:-1] + (HPC, 2, HD))
    kpart = arr[..., :, 0, :].reshape(w_slice.shape[:-1] + (HPC * HD,))
    vpart = arr[..., :, 1, :].reshape(w_slice.shape[:-1] + (HPC * HD,))
    return np.ascontiguousarray(np.concatenate([kpart, vpart], axis=-1))


def make_in_maps(x_enc, x_dec, Wq, bq, Wkv, bkv, Wo, bo, n_cores=8,
                 HPC=8, HD=64):
    import ml_dtypes
    b16 = ml_dtypes.bfloat16
    CPC = HPC * HD
    in_maps = []
    xet = [np.ascontiguousarray(x_enc[b].T).astype(b16)
           for b in range(x_enc.shape[0])]
    xdt = [np.ascontiguousarray(x_dec[b].T).astype(b16)
           for b in range(x_dec.shape[0])]
    for c in range(n_cores):
        b, hg = c // 2, c % 2
        wkv_slice = Wkv[:, hg * 2 * CPC:(hg + 1) * 2 * CPC]
        bkv_slice = bkv[hg * 2 * CPC:(hg + 1) * 2 * CPC]
        in_maps.append({
            "x_enc_t": xet[b],
            "x_dec_t": xdt[b],
            "wq": np.ascontiguousarray(
                Wq[:, hg * CPC:(hg + 1) * CPC]).astype(b16),
            "wkv_g": _group_kv_cols(wkv_slice, HPC, HD).astype(b16),
            "wo": np.ascontiguousarray(
                Wo[hg * CPC:(hg + 1) * CPC, :]).astype(b16),
            "bq": np.ascontiguousarray(bq[hg * CPC:(hg + 1) * CPC]),
            "bkv_g": _group_kv_cols(bkv_slice, HPC, HD),
            "bo": np.ascontiguousarray(bo) if hg == 0 else np.zeros_like(bo),
        })
    return in_maps


def kernel(x_enc, x_dec, mask, Wq, bq, Wkv, bkv, Wo, bo):
    x_enc = np.asarray(x_enc, dtype=np.float32)
    x_dec = np.asarray(x_dec, dtype=np.float32)
    Wq = np.asarray(Wq, dtype=np.float32)
    bq = np.asarray(bq, dtype=np.float32)
    Wkv = np.asarray(Wkv, dtype=np.float32)
    bkv = np.asarray(bkv, dtype=np.float32)
    Wo = np.asarray(Wo, dtype=np.float32)
    bo = np.asarray(bo, dtype=np.float32)
    mask = np.asarray(mask)
    if mask.any():
        raise ValueError("kernel assumes a zero additive mask (spec fill=zeros)")

    nc = _get_nc()
    in_maps = make_in_maps(x_enc, x_dec, Wq, bq, Wkv, bkv, Wo, bo)
    res = run_bass_kernel_spmd(nc, in_maps, core_ids=list(range(8)))
    outs = [res.results[c]["out"] for c in range(8)]
    B = x_enc.shape[0]
    full = np.stack([outs[2 * b] + outs[2 * b + 1] for b in range(B)], axis=0)
    return full


if __name__ == "__main__":
    import time
    t0 = time.time()
    nc = _get_nc()
    print(f"build+compile ok in {time.time() - t0:.1f}s")


# revision 45
# speedup vs baseline: 1.0002x; 1.0002x over previous
"""Multi-head cross-attention on Trainium2, 8-core SPMD.

Problem (hardcoded): B=4, T=2048, D=1024, H=16 heads, head_dim=64, fp32.
    kv = x_enc @ Wkv + bkv ; q = x_dec @ Wq + bq
    per head: S = q_h k_h^T / sqrt(64); P = softmax(S + mask); O_h = P v_h
    out = concat_h(O_h) @ Wo + bo

Sharding: data parallel over batch (4 slices x 2 cores each) and tensor
parallel over heads within each pair (8 heads per core).  Each core
computes a partial output  Y_local @ Wo[rows_local]  (+bo on the even
core of the pair); the host unshards by summing each pair's partials
and stacking the 4 batch slices.  The host pre-transposes the
activations, regroups Wkv columns, and converts x/W inputs to bf16
(all matmul operands run bf16 with fp32 PSUM accumulation; measured
rel err ~4e-3 vs the 2e-2 gate).

The mask input is structurally zero for this problem (spec fill
"zeros"); softmax(S + 0) == softmax(S), so the kernel does not load it
(checked on the host).

Per-core schedule — one fused, software-pipelined stream:
  * Pre-attention (shortest path to starting the ACT exp stream):
    K^T for pair 0, V for all k-tiles in NATURAL [k, d] layout
    (stationary = x_enc^T chunks; per-head [k, 65] layout whose 65th
    column is a constant 1 -> softmax denominators fall out of the PV
    matmul for free), and Q^T for q-chunk 0.
  * Attention: ONE flat pipeline over all 256 (q-chunk, pair, k-tile)
    steps: S^T tile = (K^T slice)^T Q^T slice per parity (64-partition
    contraction, no padding); P^T = exp(S^T/8) on ACT ([128,1024] per
    instruction, bf16 out) lagging mm1 by 2; O'^T += V_aug^T P^T
    lagging exp by 2 (pt ring depth 5) so the PE never waits on the
    exp just issued.  The attention body is ACT-bound (~1.0us/step);
    all remaining work drips into the PE stream as filler:
      - the remaining K^T/Q^T projection matmuls (one per step,
        force-drained before any consumer so program order is safe),
      - the previous q-chunk's output projection (one matmul per odd
        step; Y^T as stationary vs Wo tiles, +bo).
  * Pair-boundary normalize, off the critical path: the PSUM
    accumulators are copied to SBUF scratch immediately (fast free),
    the denominator row is DMA'd to partition 0, reciprocal on DVE
    (approx, 18 bits), partition-broadcast on the otherwise-idle
    GpSimd engine, multiply into the pair-stacked Y^T tile (odd
    parity partition-shifted via SBUF->SBUF DMA).
  PSUM: S^T 2x[128,1024] (4 banks) + PV accumulators (2) + shared
  aux ring for projections/out-projection (2) = 8 banks exactly.
"""

import numpy as np

import concourse.bass as bass
import concourse.mybir as mybir
import concourse.tile as tile
from concourse import bacc
from concourse.bass_utils import run_bass_kernel_spmd
from concourse.masks import make_identity

f32 = mybir.dt.float32
f32r = mybir.dt.float32r
bf16 = mybir.dt.bfloat16
AF = mybir.ActivationFunctionType
ALU = mybir.AluOpType

P = 128


def build_nc(T=2048, D=1024, HPC=8, HD=64, n_cores=8, use_f32r=True):
    """Build + compile the per-core Bass program. HPC = heads per core."""
    assert HD == 64 and HPC % 2 == 0 and T % 512 == 0 and D % P == 0
    CPC = HPC * HD          # q/out channels per core (512)
    TC = 512                # token chunk (psum free dim), phases A/B
    QC = 512                # q chunk width, attention phase
    NQ = T // TC            # token chunks (4)
    ND = D // P             # model-dim chunks (8)
    NG = HPC // 2           # head pairs (4)
    NKT = T // P            # k-token tiles (16)
    NQC = T // QC           # q chunks (4)
    HD1 = HD + 1            # V columns + ones column (65)
    HD2 = HD + 4            # padded to 68 cols: 8-byte-aligned bf16
                            # stationary slices (65*2B is 2-byte aligned
                            # and measurably slows LDWEIGHTS)
    VH = NKT * HD2          # per-head vnat columns
    SCALE = float(1.0 / np.sqrt(HD))
    ON = 512                # out-proj free chunk
    NON = D // ON           # 2

    MDT = f32r if use_f32r else f32    # dtype of matmul-input tiles
    BDT = bf16                         # dtype of attention operands

    def rr_(ap):                       # bitcast for DMA loads from f32 DRAM
        return ap.bitcast(MDT) if use_f32r else ap

    nc = bacc.Bacc("TRN2", target_bir_lowering=False, debug=False,
                   enable_asserts=False, num_devices=n_cores)

    xeT = nc.dram_tensor("x_enc_t", [D, T], bf16, kind="ExternalInput").ap()
    xdT = nc.dram_tensor("x_dec_t", [D, T], bf16, kind="ExternalInput").ap()
    wq_d = nc.dram_tensor("wq", [D, CPC], bf16, kind="ExternalInput").ap()
    wkv_d = nc.dram_tensor("wkv_g", [D, 2 * CPC], bf16, kind="ExternalInput").ap()
    wo_d = nc.dram_tensor("wo", [CPC, D], bf16, kind="ExternalInput").ap()
    bq_d = nc.dram_tensor("bq", [CPC], f32, kind="ExternalInput").ap()
    bkv_d = nc.dram_tensor("bkv_g", [2 * CPC], f32, kind="ExternalInput").ap()
    bo_d = nc.dram_tensor("bo", [D], f32, kind="ExternalInput").ap()
    out_d = nc.dram_tensor("out", [T, D], f32, kind="ExternalOutput").ap()

    with tile.TileContext(nc) as tc:
      with tc.tile_pool(name="const", bufs=1) as cpool:
        # identity (ones source) + ones rows for PE broadcasts: row 0
        # (bias rows at partition 0); f32r producers must be DVE ops
        ident = cpool.tile([P, P], f32, name="ident")
        make_identity(nc, ident)
        ones_t = cpool.tile([P, P], f32, name="ones_t")
        nc.vector.tensor_scalar(ones_t[0:1, :], ident[0:1, :],
                                0.0, 1.0, ALU.mult, ALU.add)

        bo_row = cpool.tile([1, D], f32, name="bo_row")
        bkv_k_sb = cpool.tile([P, NG], f32, name="bkv_k_sb")
        bq_sb = cpool.tile([P, NG], f32, name="bq_sb")
        bkv_v_row = cpool.tile([1, CPC], f32, name="bkv_v_row")
        nc.sync.dma_start(out=bo_row[:], in_=bo_d[:].unsqueeze(0))
        nc.sync.dma_start(out=bkv_v_row[:],
                          in_=bkv_d[CPC:2 * CPC].unsqueeze(0))

        # persistent activations
        kT = [cpool.tile([P, T], BDT, name=f"kT{g}") for g in range(NG)]
        qT = [cpool.tile([P, T], BDT, name=f"qT{g}") for g in range(NG)]
        # V natural, per head h: [k-token partitions, NKT*(HD+1)];
        # column kt*65+64 is constant 1.0 (softmax denominator trick)
        vnat = cpool.tile([P, HPC * VH], BDT, name="vnat")
        vnat3 = vnat[:].rearrange("p (h c) -> p h c", h=HPC)

        def vn(h):                      # per-head view [P, VH]
            return vnat3[:, h, :]

        wo_sb = [cpool.tile([P, D], BDT, name=f"wo{g}") for g in range(NG)]
        bo_bc = cpool.tile([P, D], f32, name="bo_bc")

        # ---------- Fused projections + attention + out-projection ----------
        # One scope: V + K(g0) + Q(qc0) are produced up front (shortest
        # path to starting the ACT exp stream); the remaining K/Q
        # projection matmuls and the out-projection are drip-fed into the
        # attention step loop as PE filler behind the ACT-bound exp stream.
        with tc.tile_pool(name="work", bufs=1) as wp, \
             tc.tile_pool(name="ps_S", bufs=1, space="PSUM") as psS, \
             tc.tile_pool(name="ps_O", bufs=1, space="PSUM") as psO, \
             tc.tile_pool(name="ps_aux", bufs=1, space="PSUM") as psA:

            xcs = [wp.tile([P, T], BDT, name=f"xc{d}") for d in range(ND)]
            xds = [wp.tile([P, T], BDT, name=f"xd{d}") for d in range(ND)]
            wkv_sb = [wp.tile([P, 2 * CPC], BDT, name=f"wkv{d}")
                      for d in range(ND)]
            wq_sb = [wp.tile([P, CPC], BDT, name=f"wq{d}")
                     for d in range(ND)]

            # DMA emission order = need order: tiny bias rows, x_enc +
            # wkv (critical for the pre-C work), x_dec q-chunk 0 + wq
            # (pre-C B), then the rest.
            nc.sync.dma_start(out=bkv_v_row[:],
                              in_=bkv_d[CPC:2 * CPC].unsqueeze(0))
            nc.sync.dma_start(out=bo_row[:], in_=bo_d[:].unsqueeze(0))
            for d in range(ND):      # wkv first: every group needs all d
                nc.sync.dma_start(out=wkv_sb[d][:, 0:CPC],
                                  in_=wkv_d[d * P:(d + 1) * P, 0:CPC])
                nc.sync.dma_start(out=wkv_sb[d][:, CPC:2 * CPC],
                                  in_=wkv_d[d * P:(d + 1) * P, CPC:2 * CPC])
            for c in range(4):       # x_enc in consumption order (chunk 0
                for d in range(ND):  # of every d-slice first)
                    cc = slice(c * (T // 4), (c + 1) * (T // 4))
                    nc.sync.dma_start(out=xcs[d][:, cc],
                                      in_=xeT[d * P:(d + 1) * P, cc])
            for g in range(NG):
                nc.sync.dma_start(out=bkv_k_sb[:, g:g + 1],
                                  in_=bkv_d[g * P:(g + 1) * P].unsqueeze(1))
                nc.sync.dma_start(out=bq_sb[:, g:g + 1],
                                  in_=bq_d[g * P:(g + 1) * P].unsqueeze(1))
            for d in range(ND):
                nc.sync.dma_start(out=xds[d][:, 0:TC],
                                  in_=xdT[d * P:(d + 1) * P, 0:TC])
                nc.sync.dma_start(out=wq_sb[d][:],
                                  in_=wq_d[d * P:(d + 1) * P, :])
            for d in range(ND):
                for c in range(1, 4):
                    cc = slice(c * (T // 4), (c + 1) * (T // 4))
                    nc.sync.dma_start(out=xds[d][:, cc],
                                      in_=xdT[d * P:(d + 1) * P, cc])
            for g in range(NG):
                nc.sync.dma_start(out=wo_sb[g][:, 0:CPC],
                                  in_=wo_d[g * P:(g + 1) * P, 0:CPC])
                nc.sync.dma_start(out=wo_sb[g][:, CPC:D],
                                  in_=wo_d[g * P:(g + 1) * P, CPC:D])

            # bias broadcast rows -> [128, *] (PE ones-matmul, fp32, tiny)
            bias_vbc = wp.tile([P, CPC], f32, name="bias_vbc")
            ps_vb = psA.tile([P, CPC], f32, tag="aux", bufs=2, name="ps_vb")
            nc.tensor.matmul(ps_vb[:], ones_t[0:1, :], bkv_v_row[0:1, :],
                             skip_group_check=True)
            nc.vector.tensor_copy(bias_vbc[:], ps_vb[:])
            # constant-1 denominator column (+ pad cols) of every V tile
            nc.vector.tensor_scalar(
                vnat[:].rearrange("p (h k c) -> p h k c", h=HPC, k=NKT)
                [:, :, :, HD:HD2],
                bias_vbc[:].rearrange("p (a b c) -> p a b c", a=HPC, b=NKT),
                0.0, 1.0, ALU.mult, ALU.add)
            for o in range(NON):
                ocols = slice(o * ON, (o + 1) * ON)
                ps_bo = psA.tile([P, ON], f32, tag="aux", bufs=2,
                                 name=f"psbo_{o}")
                nc.tensor.matmul(ps_bo[:], ones_t[0:1, :],
                                 bo_row[0:1, ocols], skip_group_check=True)
                nc.vector.tensor_copy(bo_bc[:, ocols], ps_bo[:])

            # ---- projection emitters (used dense pre-C, dripped in C) ----
            prj = {}

            def proj_mm(kind, g, tq, d):
                """One projection matmul: kind 'k' -> kT[g], 'q' -> qT[g]."""
                tcols = slice(tq * TC, (tq + 1) * TC)
                if d == 0:
                    prj[(kind, g, tq)] = psA.tile(
                        [P, TC], f32, tag="aux", bufs=2,
                        name=f"p{kind}_{g}_{tq}")
                pp = prj[(kind, g, tq)]
                if kind == "k":
                    nc.tensor.matmul(pp[:], wkv_sb[d][:, g * P:(g + 1) * P],
                                     xcs[d][:, tcols],
                                     start=(d == 0), stop=(d == ND - 1),
                                     skip_group_check=True)
                else:
                    nc.tensor.matmul(pp[:], wq_sb[d][:, g * P:(g + 1) * P],
                                     xds[d][:, tcols],
                                     start=(d == 0), stop=(d == ND - 1),
                                     skip_group_check=True)
                if d == ND - 1:
                    dst = kT[g] if kind == "k" else qT[g]
                    bias = bkv_k_sb if kind == "k" else bq_sb
                    nc.vector.tensor_scalar_add(
                        dst[:, tcols], prj.pop((kind, g, tq))[:],
                        bias[:, g:g + 1])

            vps = {}

            def v_mm(kt, d, tag="po"):
                """One V-natural matmul for 128-token tile kt (+evict)."""
                if d == 0:
                    pool = psO if tag == "po" else psA
                    vps[kt] = pool.tile([P, CPC], f32, tag=tag, bufs=2,
                                        name=f"pv_{kt}")
                pv = vps[kt]
                nc.tensor.matmul(
                    pv[:], xcs[d][:, kt * P:(kt + 1) * P],
                    wkv_sb[d][:, CPC:2 * CPC],
                    start=(d == 0), stop=(d == ND - 1),
                    skip_group_check=True)
                if d == ND - 1:
                    out3 = vnat[:].rearrange("p (h k c) -> p h k c",
                                             h=HPC, k=NKT)[:, :, kt, 0:HD]
                    nc.vector.tensor_tensor(
                        out3,
                        vps.pop(kt)[:].rearrange("p (h c) -> p h c", h=HPC),
                        bias_vbc[:].rearrange("p (h c) -> p h c", h=HPC),
                        ALU.add)

            emitted = set()

            # ---- pre-C: K(g0), V (all k-tiles), Q(qc0, all pairs) ----
            for tq in range(NQ):
                for d in range(ND):
                    proj_mm("k", 0, tq, d)
                emitted.add(("k", 0, tq))
                for i in range(TC // P):
                    kt = tq * (TC // P) + i
                    for d in range(ND):
                        v_mm(kt, d)
                    emitted.add(("v", kt, None))
            for g in range(NG):
                for d in range(ND):
                    proj_mm("q", g, 0, d)
                emitted.add(("q", g, 0))

            # ---- injected work queue (deadline order) ----
            inj = []

            def add_group(item8):
                kind, a, b = item8
                for d in range(ND):
                    inj.append((kind, a, b, d))

            for g in range(1, NG):
                for tq in range(NQ):
                    add_group(("k", g, tq))
            for tq in range(1, NQ):
                for g in range(NG):
                    add_group(("q", g, tq))

            # flat pipeline state
            steps = [(qc, g, kt)
                     for qc in range(NQC) for g in range(NG)
                     for kt in range(NKT)]
            NS = len(steps)
            pss = {}      # step idx -> psS tile
            pts = {}      # step idx -> pt tile
            pos = {}      # (qc, g) -> [po_e, po_o]
            yts = {}      # qc -> list of yT tiles per g
            osts = {}     # (qc, qt) -> ost tile
            pouts = {}

            def mm1_h(i, h2):
                qc, g, kt = steps[i]
                if h2 == 0:
                    force("k", g, kt // (TC // P))
                    force("q", g, qc)
                    pss[i] = psS.tile([P, 2 * QC], f32, tag="ps", bufs=2,
                                      name=f"ps_{i}")
                qcols = slice(qc * QC, (qc + 1) * QC)
                nc.tensor.matmul(
                    pss[i][:, h2 * QC:(h2 + 1) * QC],
                    kT[g][h2 * HD:(h2 + 1) * HD, kt * P:(kt + 1) * P],
                    qT[g][h2 * HD:(h2 + 1) * HD, qcols],
                    skip_group_check=True)

            def mm1(i):
                mm1_h(i, 0)
                mm1_h(i, 1)

            def do_exp(i):
                pt = wp.tile([P, 2 * QC], BDT, tag="pt", bufs=6,
                             name=f"pt_{i}")
                nc.scalar.activation(pt[:], pss.pop(i)[:], AF.Exp,
                                     scale=SCALE)
                pts[i] = pt

            def mm2_h(i, h2):
                qc, g, kt = steps[i]
                if h2 == 0:
                    force("v", kt, None)
                    if kt == 0:
                        pos[(qc, g)] = [
                            psO.tile([HD2, QC], f32, tag="po", bufs=2,
                                     name=f"po_{qc}_{g}_{hh}")
                            for hh in range(2)]
                po = pos[(qc, g)]
                pt = pts[i]
                nc.tensor.matmul(
                    po[h2][0:HD2, :],
                    vn(2 * g + h2)[:, kt * HD2:(kt + 1) * HD2],
                    pt[:, h2 * QC:(h2 + 1) * QC],
                    start=(kt == 0), stop=(kt == NKT - 1),
                    skip_group_check=True)
                if h2 == 1:
                    pts.pop(i)

            def mm2(i):
                mm2_h(i, 0)
                mm2_h(i, 1)

            def normalize(qc, g):
                """Evict+normalize pair (qc, g): frees po fast via SBUF
                scratch copies, then reciprocal/broadcast/mul off the
                critical path."""
                if g == 0:
                    yts[qc] = [wp.tile([P, QC], BDT, tag=f"yT{gg}", bufs=3,
                                       name=f"yT_{qc}_{gg}")
                               for gg in range(NG)]
                po = pos.pop((qc, g))
                scr = [wp.tile([HD1, QC], f32, tag=f"scr{h2}", bufs=1,
                               name=f"scr_{qc}_{g}_{h2}") for h2 in range(2)]
                # fast po eviction (DVE), even parity first
                for h2 in range(2):
                    nc.vector.tensor_copy(scr[h2][:], po[h2][0:HD1, :])
                # GpSimd partition_broadcast reads its source on Q7 core 0
                # (partitions 0-15 only): move the denominator row from
                # partition 64 to partition 0 via a tiny SBUF->SBUF DMA
                den = [wp.tile([1, QC], f32, tag=f"den{h2}", bufs=1,
                               name=f"den_{qc}_{g}_{h2}") for h2 in range(2)]
                rr = [wp.tile([1, QC], f32, tag=f"rr{h2}", bufs=1,
                              name=f"rr_{qc}_{g}_{h2}") for h2 in range(2)]
                for h2 in range(2):
                    nc.sync.dma_start(out=den[h2][0:1, :],
                                      in_=scr[h2][HD:HD1, :])
                for h2 in range(2):
                    nc.vector.reciprocal_approx_fast(
                        out=rr[h2][0:1, :], in_=den[h2][0:1, :])
                # broadcast the reciprocal row to 64 partitions on the (idle)
                # GpSimd engine; keeps the whole chain off PE/PSUM
                rbc = [wp.tile([HD, QC], f32, tag=f"rbc{h2}", bufs=1,
                               name=f"rbc_{qc}_{g}_{h2}") for h2 in range(2)]
                for h2 in range(2):
                    nc.gpsimd.partition_broadcast(
                        rbc[h2][:], rr[h2][0:1, :], channels=HD)
                # even parity -> yT rows 0:64 directly
                nc.vector.tensor_tensor(
                    yts[qc][g][0:HD, :],
                    scr[0][0:HD, :], rbc[0][:], ALU.mult)
                # odd parity -> staging tile, DMA shift to rows 64:128
                stg = wp.tile([HD, QC], BDT, tag="stg", bufs=2,
                              name=f"stg_{qc}_{g}")
                nc.vector.tensor_tensor(
                    stg[:],
                    scr[1][0:HD, :], rbc[1][:], ALU.mult)
                nc.sync.dma_start(out=yts[qc][g][HD:P, :], in_=stg[:])

            def outproj_mm(qc, qt, o, g):
                """One out-projection matmul (of an NG-deep accumulation
                group) for q-subtile qt, column chunk o, of q-chunk qc."""
                ocols = slice(o * ON, (o + 1) * ON)
                if o == 0 and g == 0:
                    osts[(qc, qt)] = wp.tile([P, D], f32, tag="ost", bufs=1,
                                             name=f"ost_{qc}_{qt}")
                if g == 0:
                    pouts[(qc, qt, o)] = psA.tile([P, ON], f32, tag="aux",
                                                  bufs=2,
                                                  name=f"pout_{qc}_{qt}_{o}")
                pout = pouts[(qc, qt, o)]
                nc.tensor.matmul(
                    pout[:], yts[qc][g][:, qt * P:(qt + 1) * P],
                    wo_sb[g][:, ocols],
                    start=(g == 0), stop=(g == NG - 1),
                    skip_group_check=True)
                if g == NG - 1:
                    ost = osts[(qc, qt)]
                    nc.vector.tensor_add(ost[:, ocols],
                                         pouts.pop((qc, qt, o))[:],
                                         bo_bc[:, ocols])
                    if o == NON - 1:
                        row0 = (qc * (QC // P) + qt) * P
                        nc.sync.dma_start(out=out_d[row0:row0 + P, :],
                                          in_=osts.pop((qc, qt))[:])

            def emit_inj(it):
                if it[0] == "v":
                    v_mm(it[1], it[3], tag="aux")
                elif it[0] in ("k", "q"):
                    proj_mm(it[0], it[1], it[2], it[3])
                else:
                    outproj_mm(*it[1:])
                if it[0] in ("k", "q", "v") and it[3] == ND - 1:
                    emitted.add((it[0], it[1], it[2]))

            def force(kind, a, b):
                """Emit queued work until group (kind, a, b) is complete —
                guarantees producers precede consumers in program order."""
                while (kind, a, b) not in emitted:
                    emit_inj(inj.pop(0))

            def drain(kt):
                # even steps: one projection matmul (deadline misses are
                # repaired by force()); odd steps: one out-projection
                # matmul.  Keeps out-projection from queueing up behind
                # the projections and landing in the drain tail.
                if inj and (inj[0][0] in ("k", "q", "v") or kt % 2 == 1):
                    emit_inj(inj.pop(0))

            def post_mm2(j):
                qc, g, kt = steps[j]
                if kt == NKT - 1:
                    normalize(qc, g)
                    if g == NG - 1:
                        for qt in range(QC // P):
                            for o in range(NON):
                                for gg in range(NG):
                                    inj.append(("o", qc, qt, o, gg))

            # mm2 lags exp by 2 steps (pt ring depth 4) so the PE never
            # waits on the exp just issued; mm1 leads by 3.
            mm1(0)
            mm1(1)
            do_exp(0)
            mm1(2)
            do_exp(1)
            for i in range(NS):
                if i >= 1:
                    mm2(i - 1)
                if i + 3 < NS:
                    mm1(i + 3)
                if i + 2 < NS:
                    do_exp(i + 2)
                if i >= 1:
                    post_mm2(i - 1)
                drain(steps[i][2])
            mm2(NS - 1)
            post_mm2(NS - 1)
            while inj:
                emit_inj(inj.pop(0))

    nc.compile()
    return nc


# ---------------------------------------------------------------------------
# Host side: sharding, run, unshard
# ---------------------------------------------------------------------------

_NC_CACHE = {}


def _get_nc():
    key = "full"
    if key not in _NC_CACHE:
        _NC_CACHE[key] = build_nc()
    return _NC_CACHE[key]


def _group_kv_cols(w_slice, HPC, HD):
    """Reorder kv columns [h-major, (k|v), d] -> K head-pair groups then V."""
    last = w_slice.shape[-1]
    assert last == HPC * 2 * HD
    arr = w_slice.reshape(w_slice.shape[:-1] + (HPC, 2, HD))
    kpart = arr[..., :, 0, :].reshape(w_slice.shape[:-1] + (HPC * HD,))
    vpart = arr[..., :, 1, :].reshape(w_slice.shape[:-1] + (HPC * HD,))
    return np.ascontiguousarray(np.concatenate([kpart, vpart], axis=-1))


def make_in_maps(x_enc, x_dec, Wq, bq, Wkv, bkv, Wo, bo, n_cores=8,
                 HPC=8, HD=64):
    import ml_dtypes
    b16 = ml_dtypes.bfloat16
    CPC = HPC * HD
    in_maps = []
    xet = [np.ascontiguousarray(x_enc[b].T).astype(b16)
           for b in range(x_enc.shape[0])]
    xdt = [np.ascontiguousarray(x_dec[b].T).astype(b16)
           for b in range(x_dec.shape[0])]
    for c in range(n_cores):
        b, hg = c // 2, c % 2
        wkv_slice = Wkv[:, hg * 2 * CPC:(hg + 1) * 2 * CPC]
        bkv_slice = bkv[hg * 2 * CPC:(hg + 1) * 2 * CPC]
        in_maps.append({
            "x_enc_t": xet[b],
            "x_dec_t": xdt[b],
            "wq": np.ascontiguousarray(
                Wq[:, hg * CPC:(hg + 1) * CPC]).astype(b16),
            "wkv_g": _group_kv_cols(wkv_slice, HPC, HD).astype(b16),
            "wo": np.ascontiguousarray(
                Wo[hg * CPC:(hg + 1) * CPC, :]).astype(b16),
            "bq": np.ascontiguousarray(bq[hg * CPC:(hg + 1) * CPC]),
            "bkv_g": _group_kv_cols(bkv_slice, HPC, HD),
            "bo": np.ascontiguousarray(bo) if hg == 0 else np.zeros_like(bo),
        })
    return in_maps


def kernel(x_enc, x_dec, mask, Wq, bq, Wkv, bkv, Wo, bo):
    x_enc = np.asarray(x_enc, dtype=np.float32)
    x_dec = np.asarray(x_dec, dtype=np.float32)
    Wq = np.asarray(Wq, dtype=np.float32)
    bq = np.asarray(bq, dtype=np.float32)
    Wkv = np.asarray(Wkv, dtype=np.float32)
    bkv = np.asarray(bkv, dtype=np.float32)
    Wo = np.asarray(Wo, dtype=np.float32)
    bo = np.asarray(bo, dtype=np.float32)
    mask = np.asarray(mask)
    if mask.any():
        raise ValueError("kernel assumes a zero additive mask (spec fill=zeros)")

    nc = _get_nc()
    in_maps = make_in_maps(x_enc, x_dec, Wq, bq, Wkv, bkv, Wo, bo)
    res = run_bass_kernel_spmd(nc, in_maps, core_ids=list(range(8)))
    outs = [res.results[c]["out"] for c in range(8)]
    B = x_enc.shape[0]
    full = np.stack([outs[2 * b] + outs[2 * b + 1] for b in range(B)], axis=0)
    return full


if __name__ == "__main__":
    import time
    t0 = time.time()
    nc = _get_nc()
    print(f"build+compile ok in {time.time() - t0:.1f}s")
